# revision 8
# baseline (speedup 1.0000x reference)
"""Trainium2 Bass kernel for nn_BandSplitDCTFilter.

Math: the reference's mirror-FFT DCT / band filter / inverse collapses to
    out_c = C1 (Z_c) C2^T - S1 (Z_c) S2^T,   Z_c = (A x_c A^T) .* W_eff_c
with A[k,j] = 2cos(pi k (2j+1)/128); C2/S2 carry the irfft half-spectrum
weights u_l and the 1/(4HW) scale; W_eff = pad(W_low)+pad(W_mid)+W_high
merges the three bands (they share the inverse basis under zero-padding).
Then y = x_out @ proj_w^T and LayerNorm.

Sharding: pure data-parallel, one sample per core (B=8 = 8 cores), small
weights replicated.

Per-core pipeline (v9). 64-row tensors are "packed": free dim split in
half across partition ranges [0:64) and [64:128) so every engine op and
PSUM tile runs 128 partitions wide. Layout pivots ride DRAM (strided
store with >=512B runs + contiguous reload) to keep the DMA instruction
count tiny — this container pays ~0.6-0.8us of sequencer time per DMA
instruction, so many small DMAs serialize.

  S2  F-h : T1[k,(w,c)]   = AT.T @ x[h,(w,c)]      (2 packed halves)
  P1  T1 -> DRAM (k,w,c order) -> T2[w,(k,c)]       (2+2 DMA instrs)
  S4  F-w : Z[l,(k,c)]    = AT.T @ T2, * W_eff fused in drain
  S5  I-l : U2s[(cs,n),(k,c)] = [C2T|S2T].T @ Z     (cos/sin stacked)
  P2  U2s -> DRAM ((cs,k),(n,c) order) -> Ustk      (2+1 DMA instrs, bf16)
  S7  I-k : per (n,chalf): X01[cj, m] = Ustk-chunk.T @ [C1T;-S1T]
  S8  proj: per 128-row tile: Y = X0c.T pjt0 + X1c.T pjt1
  S9  LN  : bn_stats/aggr + fused (y-mu)*rstd -> Yall -> 1 DMA out
Host does layout-only prep (shard/pack) and row unpermute.
"""

import numpy as np
import ml_dtypes

import bass_rust
import concourse.bass as bass
import concourse.mybir as mybir
from concourse.tile import TileContext, ScopedClock
from concourse.bass_utils import run_bass_kernel_spmd

# ---------------------------------------------------------------------------
# Workarounds: this container's walrus rejects >1 sync wait per instruction.
# ---------------------------------------------------------------------------

_wait_ctr = 0


def _split_multi_waits(nc, max_waits=1):
    global _wait_ctr
    for f in nc.m.functions:
        for bb in f.blocks:
            out = []
            dirty = False
            for ins in bb.instructions:
                si = ins.sync_info
                if si is not None and len(si.on_wait) > max_waits:
                    waits = list(si.on_wait)
                    for w in waits[:-max_waits]:
                        _wait_ctr += 1
                        nop = bass_rust.InstNoOp(name=f"I-waitsplit-{_wait_ctr}")
                        nop.engine = ins.engine
                        nop.sync_info = mybir.SyncInfo(on_wait=[w], on_update=[])
                        out.append(nop)
                    ins.sync_info = mybir.SyncInfo(
                        on_wait=waits[-max_waits:], on_update=list(si.on_update)
                    )
                    dirty = True
                out.append(ins)
            if dirty:
                bb.instructions = out


def _patched_drain_and_barrier(self, tick_clock, wait_clock):
    nc = self.nc
    probe = nc.sync.nop(nofuse=True)
    wait_clock.add_sem_waits(probe.ins, ScopedClock({None: tick_clock.global_clock}))
    si = probe.ins.sync_info
    waits = list(si.on_wait) if si is not None else []
    probe.ins.sync_info = mybir.SyncInfo(on_wait=waits[:1], on_update=[])
    name2sem = {s.name: s for s in self.sems.allocated().values()}
    for w in waits[1:]:
        nc.sync.nop(nofuse=True)._wait_ge(name2sem[w.ant_name], w.wait_value)
    nc.sync.drain()
    nc.all_engine_barrier()
    popped = nc._tile_sem_poison_stack.pop()
    assert popped is self._sem_poison
    nc.clear_and_free_semaphores(list(self.sems.allocated().values()))
    nc.all_engine_barrier()


TileContext._drain_and_barrier = _patched_drain_and_barrier

# ---------------------------------------------------------------------------

B, H, W, C = 8, 64, 64, 256
N = H * W
F32 = mybir.dt.float32
F32R = mybir.dt.float32r
BF16 = mybir.dt.bfloat16
ALU = mybir.AluOpType
ACTF = mybir.ActivationFunctionType


def _host_matrices():
    k = np.arange(64)
    j = np.arange(64)
    ang = np.pi * k[:, None] * (2 * j[None, :] + 1) / 128.0
    A = 2.0 * np.cos(ang)
    u = np.where(k == 0, 1.0, 2.0)
    C1T = np.cos(ang)
    S1T = np.sin(ang)
    C2T = u[:, None] * np.cos(ang) / 16384.0
    S2T = u[:, None] * np.sin(ang) / 16384.0

    AT = A.T.astype(np.float32)                                   # [h, k]
    kh2 = np.concatenate([AT, AT], axis=0).astype(np.float32)     # [128, 64]
    cs2_half = np.concatenate([C2T, S2T], axis=1)                 # [l, 128]
    cs2 = np.concatenate([cs2_half, cs2_half], axis=0).astype(np.float32)
    ICS = np.concatenate([C1T, -S1T], axis=0).astype(ml_dtypes.bfloat16)
    return kh2, cs2, np.ascontiguousarray(ICS)


_NC_CACHE = {}


def _build_nc(apply_gb):
    nc = bass.Bass(trn_type="TRN2")

    xa_d = nc.dram_tensor("xra", [128, 4096], F32R, kind="ExternalInput")
    xb_d = nc.dram_tensor("xrb", [128, 4096], F32R, kind="ExternalInput")
    kh_d = nc.dram_tensor("kh", [128, 64], F32R, kind="ExternalInput")
    cs_d = nc.dram_tensor("cs", [128, 128], F32R, kind="ExternalInput")
    ics_d = nc.dram_tensor("ics", [128, 64], BF16, kind="ExternalInput")
    wa_d = nc.dram_tensor("weffa", [128, 4096], F32, kind="ExternalInput")
    wb_d = nc.dram_tensor("weffb", [128, 4096], F32, kind="ExternalInput")
    pjt_d = nc.dram_tensor("pjt", [128, 512], BF16, kind="ExternalInput")
    gb_d = nc.dram_tensor("gb", [2, 256], F32, kind="ExternalInput")
    y_d = nc.dram_tensor("y", [4096, 256], F32, kind="ExternalOutput")

    with TileContext(nc) as tc:
        with (
            tc.tile_pool(name="consts", bufs=1) as consts,
            tc.tile_pool(name="wfA", bufs=1) as wfA,
            tc.tile_pool(name="wfB", bufs=1) as wfB,
            tc.tile_pool(name="sA1", bufs=1) as sA1,
            tc.tile_pool(name="sA2", bufs=1) as sA2,
            tc.tile_pool(name="sA3", bufs=1) as sA3,
            tc.tile_pool(name="sB1", bufs=1) as sB1,
            tc.tile_pool(name="sB2", bufs=1) as sB2,
            tc.tile_pool(name="sB3", bufs=1) as sB3,
            tc.tile_pool(name="zA", bufs=1) as zA,
            tc.tile_pool(name="zB", bufs=1) as zB,
            tc.tile_pool(name="dramp", bufs=1, space="DRAM") as dramp,
            tc.tile_pool(name="ps", bufs=8, space="PSUM") as ps,
            tc.tile_pool(name="small", bufs=8) as small,
        ):
            # ---- constants ----
            kh2 = consts.tile([128, 64], F32R, tag="kh2")
            cs2 = consts.tile([128, 128], F32R, tag="cs2")
            ics = consts.tile([128, 64], BF16, tag="ics")
            pjt = consts.tile([128, 512], BF16, tag="pjt")
            nc.sync.dma_start(out=kh2[:], in_=kh_d[:])
            nc.sync.dma_start(out=cs2[:], in_=cs_d[:])
            nc.sync.dma_start(out=ics[:], in_=ics_d[:])
            nc.sync.dma_start(out=pjt[:], in_=pjt_d[:])
            eps = consts.tile([128, 1], F32, tag="eps")
            nc.vector.memset(eps[:], 1e-5)
            weffA = wfA.tile([128, 4096], F32, tag="wfA")
            weffB = wfB.tile([128, 4096], F32, tag="wfB")
            nc.gpsimd.dma_start(out=weffA[:], in_=wa_d[:])
            nc.gpsimd.dma_start(out=weffB[:], in_=wb_d[:])
            if apply_gb:
                gt = consts.tile([128, 256], F32, tag="gt")
                bt = consts.tile([128, 256], F32, tag="bt")
                gb_ap = gb_d.ap()
                g_b = bass.AP(tensor=gb_ap.tensor, offset=0, ap=[[0, 128], [1, 256]])
                b_b = bass.AP(tensor=gb_ap.tensor, offset=256, ap=[[0, 128], [1, 256]])
                nc.sync.dma_start(out=gt[:], in_=g_b)
                nc.sync.dma_start(out=bt[:], in_=b_b)

            pipes = {}
            for P, (x_d, w_d, io, s1, s2, s3, zp) in enumerate(
                (
                    (xa_d, wa_d, nc.sync, sA1, sA2, sA3, zA),
                    (xb_d, wb_d, nc.scalar, sB1, sB2, sB3, zB),
                )
            ):
                # ---- S1: load (chunked so S2 starts early) ----
                X = s1.tile([128, 4096], F32R, tag=f"s{P}1")
                io.dma_start(out=X[:, 0:2048], in_=x_d[:, 0:2048])
                io.dma_start(out=X[:, 2048:4096], in_=x_d[:, 2048:4096])

                # ---- S2: F-h ----
                T1p = s2.tile([128, 4096], F32R, tag=f"s{P}2")
                for j in range(16):
                    off = 64 * (j // 8)
                    sl = slice((j % 8) * 512, (j % 8 + 1) * 512)
                    pt = ps.tile([64, 512], F32, tag="ps")
                    nc.tensor.matmul(pt[:], kh2[off:off + 64, :],
                                     X[off:off + 64, sl], start=True, stop=True)
                    eng = nc.vector.tensor_copy if j % 2 == 0 else nc.scalar.copy
                    eng(T1p[off:off + 64, sl], pt[:])

                # ---- P1: pivot via DRAM ----
                D1 = dramp.tile([64, 8192], F32R, tag=f"d1{P}")
                D1v = D1[:].rearrange("w (k c) -> k w c", c=128)
                io.dma_start(out=D1v[:, 0:32, :], in_=T1p[0:64, :])
                io.dma_start(out=D1v[:, 32:64, :], in_=T1p[64:128, :])
                T2p = s3.tile([128, 4096], F32R, tag=f"s{P}3")
                io.dma_start(out=T2p[0:64, :], in_=D1[:, 0:4096])
                io.dma_start(out=T2p[64:128, :], in_=D1[:, 4096:8192])

                # ---- S4: F-w + mask ----
                weff = weffA if P == 0 else weffB
                Zp = zp.tile([128, 4096], F32R, tag=f"z{P}")
                for j in range(16):
                    off = 64 * (j // 8)
                    sl = slice((j % 8) * 512, (j % 8 + 1) * 512)
                    pt = ps.tile([64, 512], F32, tag="ps")
                    nc.tensor.matmul(pt[:], kh2[off:off + 64, :],
                                     T2p[off:off + 64, sl], start=True, stop=True)
                    nc.vector.tensor_mul(Zp[off:off + 64, sl], pt[:],
                                         weff[off:off + 64, sl])

                # ---- S5: I-l stacked ----
                U2s = s3.tile([128, 8192], BF16, tag=f"s{P}3")
                for j in range(16):
                    off = 64 * (j // 8)
                    sl = slice((j % 8) * 512, (j % 8 + 1) * 512)
                    pt = ps.tile([128, 512], F32, tag="ps")
                    nc.tensor.matmul(pt[:], cs2[off:off + 64, :],
                                     Zp[off:off + 64, sl], start=True, stop=True)
                    dsl = slice(j * 512, (j + 1) * 512)
                    eng = nc.vector.tensor_copy if j % 2 == 0 else nc.scalar.copy
                    eng(U2s[:, dsl], pt[:])

                # ---- P2: pivot via DRAM (bf16) ----
                D2 = dramp.tile([128, 8192], BF16, tag=f"d2{P}")
                for cshalf in range(2):
                    dst = D2[cshalf * 64:(cshalf + 1) * 64, :].rearrange(
                        "k (n c) -> n k c", c=128
                    )
                    io.dma_start(out=dst,
                                 in_=U2s[cshalf * 64:(cshalf + 1) * 64, :])
                Ustk = s1.tile([128, 8192], BF16, tag=f"s{P}1")
                io.dma_start(out=Ustk[:], in_=D2[:])

                # ---- S7: I-k ----
                X01 = s2.tile([128, 4096], BF16, tag=f"s{P}2")
                for g in range(8):
                    pt = ps.tile([128, 512], F32, tag="ps")
                    for nn in range(8):
                        t = 8 * g + nn
                        nc.tensor.matmul(
                            pt[:, nn * 64:(nn + 1) * 64],
                            Ustk[:, t * 128:(t + 1) * 128],
                            ics[:], start=True, stop=True,
                        )
                    eng = nc.vector.tensor_copy if g % 2 == 0 else nc.scalar.copy
                    eng(X01[:, g * 512:(g + 1) * 512], pt[:])
                pipes[P] = X01

            # ---- S8 + S9: proj, LayerNorm -> Yall ----
            X01A, X01B = pipes[0], pipes[1]
            Yall = zA.tile([128, 8192], F32, tag="z0")
            for t2 in range(32):
                pty = ps.tile([128, 256], F32, tag="ps")
                nc.tensor.matmul(pty[:], X01A[:, t2 * 128:(t2 + 1) * 128],
                                 pjt[:, 0:256], start=True, stop=False)
                nc.tensor.matmul(pty[:], X01B[:, t2 * 128:(t2 + 1) * 128],
                                 pjt[:, 256:512], start=False, stop=True)
                stats = small.tile([128, 6], F32, tag="stats")
                mv = small.tile([128, 2], F32, tag="mv")
                nc.vector.bn_stats(out=stats[:], in_=pty[:])
                nc.vector.bn_aggr(out=mv[:], in_=stats[:])
                rstd = small.tile([128, 1], F32, tag="rstd")
                negmu = small.tile([128, 1], F32, tag="negmu")
                nc.scalar.activation(out=rstd[:], in_=mv[:, 1:2], func=ACTF.Sqrt,
                                     bias=eps[:], scale=1.0)
                nc.vector.reciprocal(rstd[:], rstd[:])
                nc.vector.tensor_scalar_mul(negmu[:], mv[:, 0:1], -1.0)
                ysl = slice(t2 * 256, (t2 + 1) * 256)
                nc.vector.tensor_scalar(Yall[:, ysl], pty[:], negmu[:], rstd[:],
                                        op0=ALU.add, op1=ALU.mult)
                if apply_gb:
                    nc.vector.tensor_mul(Yall[:, ysl], Yall[:, ysl], gt[:])
                    nc.vector.tensor_add(Yall[:, ysl], Yall[:, ysl], bt[:])

            # ---- S10: one strided store ----
            yv = y_d[:].rearrange("(t r) d -> r t d", r=128)
            nc.scalar.dma_start(out=yv, in_=Yall[:])

    _split_multi_waits(nc)
    return nc


def _get_nc(apply_gb):
    key = bool(apply_gb)
    if key not in _NC_CACHE:
        _NC_CACHE[key] = _build_nc(key)
    return _NC_CACHE[key]


def _make_inputs(x, W_low, W_mid, W_high, proj_w, ln_g, ln_b):
    kh2, cs2, ICS = _host_matrices()

    W_eff = W_high[0].copy()
    W_eff[:32, :32] += W_mid[0]
    W_eff[:16, :16] += W_low[0]
    weffs = []
    for P in range(2):
        wr = W_eff[:, :, P * 128:(P + 1) * 128].transpose(1, 0, 2).reshape(64, 8192)
        weffs.append(np.ascontiguousarray(
            wr.reshape(64, 2, 4096).transpose(1, 0, 2).reshape(128, 4096)
        ))

    pjt = np.zeros((128, 512), ml_dtypes.bfloat16)
    pjt[:, :256] = proj_w.T[:128]
    pjt[:, 256:] = proj_w.T[128:]

    gb = np.stack([ln_g, ln_b]).astype(np.float32)
    consts = {"kh": kh2, "cs": cs2, "ics": ICS,
              "weffa": weffs[0], "weffb": weffs[1], "pjt": pjt, "gb": gb}

    in_maps = []
    for b in range(B):
        m = dict(consts)
        for P, name in ((0, "xra"), (1, "xrb")):
            xp = x[b].reshape(64, 64, 256)[:, :, P * 128:(P + 1) * 128]
            m[name] = np.ascontiguousarray(
                xp.reshape(64, 2, 32, 128).transpose(1, 0, 2, 3).reshape(128, 4096)
            )
        in_maps.append(m)
    return in_maps


def kernel(x, W_low, W_mid, W_high, proj_w, ln_g, ln_b):
    x = np.ascontiguousarray(np.asarray(x, dtype=np.float32))
    W_low = np.asarray(W_low, dtype=np.float32)
    W_mid = np.asarray(W_mid, dtype=np.float32)
    W_high = np.asarray(W_high, dtype=np.float32)
    proj_w = np.asarray(proj_w, dtype=np.float32)
    ln_g = np.asarray(ln_g, dtype=np.float32)
    ln_b = np.asarray(ln_b, dtype=np.float32)

    apply_gb = not (np.all(ln_g == 1.0) and np.all(ln_b == 0.0))
    in_maps = _make_inputs(x, W_low, W_mid, W_high, proj_w, ln_g, ln_b)
    nc = _get_nc(apply_gb)
    res = run_bass_kernel_spmd(nc, in_maps, core_ids=list(range(B)))

    out = np.empty((B, N, C), np.float32)
    for b in range(B):
        yc = res.results[b]["y"]
        out[b] = yc.reshape(64, 64, 256).transpose(1, 0, 2).reshape(4096, 256)
    return out


# revision 9
# speedup vs baseline: 1.2965x; 1.2965x over previous
"""Trainium2 Bass kernel for nn_BandSplitDCTFilter.

Math: the reference's mirror-FFT DCT / band filter / inverse collapses to
    out_c = C1 (Z_c) C2^T - S1 (Z_c) S2^T,   Z_c = (A x_c A^T) .* W_eff_c
with A[k,j] = 2cos(pi k (2j+1)/128); C2/S2 carry the irfft half-spectrum
weights u_l and the 1/(4HW) scale; W_eff = pad(W_low)+pad(W_mid)+W_high
merges the three bands (they share the inverse basis under zero-padding).
Then y = x_out @ proj_w^T and LayerNorm.

Sharding: pure data-parallel, one sample per core (B=8 = 8 cores), small
weights replicated.

Per-core pipeline (v9). 64-row tensors are "packed": free dim split in
half across partition ranges [0:64) and [64:128) so every engine op and
PSUM tile runs 128 partitions wide. Layout pivots ride DRAM (strided
store with >=512B runs + contiguous reload) to keep the DMA instruction
count tiny — this container pays ~0.6-0.8us of sequencer time per DMA
instruction, so many small DMAs serialize.

  S2  F-h : T1[k,(w,c)]   = AT.T @ x[h,(w,c)]      (2 packed halves)
  P1  T1 -> DRAM (k,w,c order) -> T2[w,(k,c)]       (2+2 DMA instrs)
  S4  F-w : Z[l,(k,c)]    = AT.T @ T2, * W_eff fused in drain
  S5  I-l : U2s[(cs,n),(k,c)] = [C2T|S2T].T @ Z     (cos/sin stacked)
  P2  U2s -> DRAM ((cs,k),(n,c) order) -> Ustk      (2+1 DMA instrs, bf16)
  S7  I-k : per (n,chalf): X01[cj, m] = Ustk-chunk.T @ [C1T;-S1T]
  S8  proj: per 128-row tile: Y = X0c.T pjt0 + X1c.T pjt1
  S9  LN  : bn_stats/aggr + fused (y-mu)*rstd -> Yall -> 1 DMA out
Host does layout-only prep (shard/pack) and row unpermute.
"""

import numpy as np
import ml_dtypes

import bass_rust
import concourse.bass as bass
import concourse.mybir as mybir
from concourse.tile import TileContext, ScopedClock
from concourse.bass_utils import run_bass_kernel_spmd

# ---------------------------------------------------------------------------
# Workarounds: this container's walrus rejects >1 sync wait per instruction.
# ---------------------------------------------------------------------------

_wait_ctr = 0


def _split_multi_waits(nc, max_waits=1):
    global _wait_ctr
    for f in nc.m.functions:
        for bb in f.blocks:
            out = []
            dirty = False
            for ins in bb.instructions:
                si = ins.sync_info
                if si is not None and len(si.on_wait) > max_waits:
                    waits = list(si.on_wait)
                    for w in waits[:-max_waits]:
                        _wait_ctr += 1
                        nop = bass_rust.InstNoOp(name=f"I-waitsplit-{_wait_ctr}")
                        nop.engine = ins.engine
                        nop.sync_info = mybir.SyncInfo(on_wait=[w], on_update=[])
                        out.append(nop)
                    ins.sync_info = mybir.SyncInfo(
                        on_wait=waits[-max_waits:], on_update=list(si.on_update)
                    )
                    dirty = True
                out.append(ins)
            if dirty:
                bb.instructions = out


def _patched_drain_and_barrier(self, tick_clock, wait_clock):
    nc = self.nc
    probe = nc.sync.nop(nofuse=True)
    wait_clock.add_sem_waits(probe.ins, ScopedClock({None: tick_clock.global_clock}))
    si = probe.ins.sync_info
    waits = list(si.on_wait) if si is not None else []
    probe.ins.sync_info = mybir.SyncInfo(on_wait=waits[:1], on_update=[])
    name2sem = {s.name: s for s in self.sems.allocated().values()}
    for w in waits[1:]:
        nc.sync.nop(nofuse=True)._wait_ge(name2sem[w.ant_name], w.wait_value)
    nc.sync.drain()
    nc.all_engine_barrier()
    popped = nc._tile_sem_poison_stack.pop()
    assert popped is self._sem_poison
    nc.clear_and_free_semaphores(list(self.sems.allocated().values()))
    nc.all_engine_barrier()


TileContext._drain_and_barrier = _patched_drain_and_barrier

# ---------------------------------------------------------------------------

B, H, W, C = 8, 64, 64, 256
N = H * W
F32 = mybir.dt.float32
F32R = mybir.dt.float32r
BF16 = mybir.dt.bfloat16
ALU = mybir.AluOpType
ACTF = mybir.ActivationFunctionType


def _host_matrices():
    k = np.arange(64)
    j = np.arange(64)
    ang = np.pi * k[:, None] * (2 * j[None, :] + 1) / 128.0
    A = 2.0 * np.cos(ang)
    u = np.where(k == 0, 1.0, 2.0)
    C1T = np.cos(ang)
    S1T = np.sin(ang)
    C2T = u[:, None] * np.cos(ang) / 16384.0
    S2T = u[:, None] * np.sin(ang) / 16384.0

    AT = A.T.astype(np.float32)                                   # [h, k]
    kh2 = np.concatenate([AT, AT], axis=0).astype(np.float32)     # [128, 64]
    cs2_half = np.concatenate([C2T, S2T], axis=1)                 # [l, 128]
    cs2 = np.concatenate([cs2_half, cs2_half], axis=0).astype(np.float32)
    ICS = np.concatenate([C1T, -S1T], axis=0).astype(ml_dtypes.bfloat16)
    return kh2, cs2, np.ascontiguousarray(ICS)


_NC_CACHE = {}


def _build_nc(apply_gb):
    nc = bass.Bass(trn_type="TRN2")

    xa_d = nc.dram_tensor("xra", [128, 4096], F32R, kind="ExternalInput")
    xb_d = nc.dram_tensor("xrb", [128, 4096], F32R, kind="ExternalInput")
    kh_d = nc.dram_tensor("kh", [128, 64], F32R, kind="ExternalInput")
    cs_d = nc.dram_tensor("cs", [128, 128], F32R, kind="ExternalInput")
    ics_d = nc.dram_tensor("ics", [128, 64], BF16, kind="ExternalInput")
    wa_d = nc.dram_tensor("weffa", [128, 4096], F32, kind="ExternalInput")
    wb_d = nc.dram_tensor("weffb", [128, 4096], F32, kind="ExternalInput")
    pjt_d = nc.dram_tensor("pjt", [128, 512], BF16, kind="ExternalInput")
    gb_d = nc.dram_tensor("gb", [2, 256], F32, kind="ExternalInput")
    y_d = nc.dram_tensor("y", [4096, 256], F32, kind="ExternalOutput")

    with TileContext(nc) as tc:
        with (
            tc.tile_pool(name="consts", bufs=1) as consts,
            tc.tile_pool(name="wfA", bufs=1) as wfA,
            tc.tile_pool(name="wfB", bufs=1) as wfB,
            tc.tile_pool(name="sA1", bufs=1) as sA1,
            tc.tile_pool(name="sA2", bufs=1) as sA2,
            tc.tile_pool(name="sA3", bufs=1) as sA3,
            tc.tile_pool(name="sB1", bufs=1) as sB1,
            tc.tile_pool(name="sB2", bufs=1) as sB2,
            tc.tile_pool(name="sB3", bufs=1) as sB3,
            tc.tile_pool(name="zA", bufs=1) as zA,
            tc.tile_pool(name="zB", bufs=1) as zB,
            tc.tile_pool(name="dramp", bufs=1, space="DRAM") as dramp,
            tc.tile_pool(name="ps", bufs=8, space="PSUM") as ps,
            tc.tile_pool(name="small", bufs=8) as small,
        ):
            # ---- constants ----
            kh2 = consts.tile([128, 64], F32R, tag="kh2")
            cs2 = consts.tile([128, 128], F32R, tag="cs2")
            ics = consts.tile([128, 64], BF16, tag="ics")
            pjt = consts.tile([128, 512], BF16, tag="pjt")
            nc.sync.dma_start(out=kh2[:], in_=kh_d[:])
            nc.sync.dma_start(out=cs2[:], in_=cs_d[:])
            nc.sync.dma_start(out=ics[:], in_=ics_d[:])
            nc.sync.dma_start(out=pjt[:], in_=pjt_d[:])
            eps = consts.tile([128, 1], F32, tag="eps")
            nc.vector.memset(eps[:], 1e-5)
            weffA = wfA.tile([128, 4096], F32, tag="wfA")
            weffB = wfB.tile([128, 4096], F32, tag="wfB")
            nc.gpsimd.dma_start(out=weffA[:], in_=wa_d[:])
            nc.gpsimd.dma_start(out=weffB[:], in_=wb_d[:])
            if apply_gb:
                gt = consts.tile([128, 256], F32, tag="gt")
                bt = consts.tile([128, 256], F32, tag="bt")
                gb_ap = gb_d.ap()
                g_b = bass.AP(tensor=gb_ap.tensor, offset=0, ap=[[0, 128], [1, 256]])
                b_b = bass.AP(tensor=gb_ap.tensor, offset=256, ap=[[0, 128], [1, 256]])
                nc.sync.dma_start(out=gt[:], in_=g_b)
                nc.sync.dma_start(out=bt[:], in_=b_b)

            # Two channel-half pipes, emission staggered so one pipe's
            # DRAM pivots overlap the other pipe's compute.
            cfg = {
                0: dict(x_d=xa_d, io=nc.sync, s1=sA1, s2=sA2, s3=sA3, zp=zA),
                1: dict(x_d=xb_d, io=nc.scalar, s1=sB1, s2=sB2, s3=sB3, zp=zB),
            }
            st = {0: {}, 1: {}}

            def s1_load(P):
                c = cfg[P]
                X = c["s1"].tile([128, 4096], F32R, tag=f"s{P}1")
                c["io"].dma_start(out=X[:, 0:2048], in_=c["x_d"][:, 0:2048])
                c["io"].dma_start(out=X[:, 2048:4096], in_=c["x_d"][:, 2048:4096])
                st[P]["X"] = X

            def s2_fh(P):
                c = cfg[P]
                X = st[P]["X"]
                T1p = c["s2"].tile([128, 4096], F32R, tag=f"s{P}2")
                for j in range(16):
                    off = 64 * (j // 8)
                    sl = slice((j % 8) * 512, (j % 8 + 1) * 512)
                    pt = ps.tile([64, 512], F32, tag="ps")
                    nc.tensor.matmul(pt[:], kh2[off:off + 64, :],
                                     X[off:off + 64, sl], start=True, stop=True)
                    eng = nc.vector.tensor_copy if j % 2 == 0 else nc.scalar.copy
                    eng(T1p[off:off + 64, sl], pt[:])
                st[P]["T1p"] = T1p

            def p1_pivot(P):
                c = cfg[P]
                T1p = st[P]["T1p"]
                D1 = dramp.tile([64, 8192], F32R, tag=f"d1{P}")
                D1v = D1[:].rearrange("w (k c) -> k w c", c=128)
                c["io"].dma_start(out=D1v[:, 0:32, :], in_=T1p[0:64, :])
                c["io"].dma_start(out=D1v[:, 32:64, :], in_=T1p[64:128, :])
                T2p = c["s3"].tile([128, 4096], F32R, tag=f"s{P}3")
                c["io"].dma_start(out=T2p[0:64, :], in_=D1[:, 0:4096])
                c["io"].dma_start(out=T2p[64:128, :], in_=D1[:, 4096:8192])
                st[P]["T2p"] = T2p

            def s4_s5(P):
                c = cfg[P]
                T2p = st[P]["T2p"]
                weff = weffA if P == 0 else weffB
                Zp = c["zp"].tile([128, 4096], F32R, tag=f"z{P}")
                for j in range(16):
                    off = 64 * (j // 8)
                    sl = slice((j % 8) * 512, (j % 8 + 1) * 512)
                    pt = ps.tile([64, 512], F32, tag="ps")
                    nc.tensor.matmul(pt[:], kh2[off:off + 64, :],
                                     T2p[off:off + 64, sl], start=True, stop=True)
                    nc.vector.tensor_mul(Zp[off:off + 64, sl], pt[:],
                                         weff[off:off + 64, sl])
                U2s = c["s3"].tile([128, 8192], BF16, tag=f"s{P}3")
                for j in range(16):
                    off = 64 * (j // 8)
                    sl = slice((j % 8) * 512, (j % 8 + 1) * 512)
                    pt = ps.tile([128, 512], F32, tag="ps")
                    nc.tensor.matmul(pt[:], cs2[off:off + 64, :],
                                     Zp[off:off + 64, sl], start=True, stop=True)
                    dsl = slice(j * 512, (j + 1) * 512)
                    eng = nc.vector.tensor_copy if j % 2 == 0 else nc.scalar.copy
                    eng(U2s[:, dsl], pt[:])
                st[P]["U2s"] = U2s

            def p2_pivot(P):
                c = cfg[P]
                U2s = st[P]["U2s"]
                D2 = dramp.tile([128, 8192], BF16, tag=f"d2{P}")
                for cshalf in range(2):
                    dst = D2[cshalf * 64:(cshalf + 1) * 64, :].rearrange(
                        "k (n c) -> n k c", c=128
                    )
                    c["io"].dma_start(out=dst,
                                      in_=U2s[cshalf * 64:(cshalf + 1) * 64, :])
                Ustk = c["s1"].tile([128, 8192], BF16, tag=f"s{P}1")
                c["io"].dma_start(out=Ustk[:], in_=D2[:])
                st[P]["Ustk"] = Ustk

            def s7_ik(P):
                c = cfg[P]
                Ustk = st[P]["Ustk"]
                X01 = c["s2"].tile([128, 4096], BF16, tag=f"s{P}2")
                for g in range(8):
                    pt = ps.tile([128, 512], F32, tag="ps")
                    for nn in range(8):
                        t = 8 * g + nn
                        nc.tensor.matmul(
                            pt[:, nn * 64:(nn + 1) * 64],
                            Ustk[:, t * 128:(t + 1) * 128],
                            ics[:], start=True, stop=True,
                        )
                    eng = nc.vector.tensor_copy if g % 2 == 0 else nc.scalar.copy
                    eng(X01[:, g * 512:(g + 1) * 512], pt[:])
                st[P]["X01"] = X01

            s1_load(0)
            s1_load(1)
            s2_fh(0)
            p1_pivot(0)
            s2_fh(1)
            s4_s5(0)
            p1_pivot(1)
            p2_pivot(0)
            s4_s5(1)
            s7_ik(0)
            p2_pivot(1)
            s7_ik(1)
            X01A, X01B = st[0]["X01"], st[1]["X01"]

            # ---- S8 + S9: proj, LayerNorm -> Yall ----
            Yall = zA.tile([128, 8192], F32, tag="z0")
            for t2 in range(32):
                pty = ps.tile([128, 256], F32, tag="ps")
                nc.tensor.matmul(pty[:], X01A[:, t2 * 128:(t2 + 1) * 128],
                                 pjt[:, 0:256], start=True, stop=False)
                nc.tensor.matmul(pty[:], X01B[:, t2 * 128:(t2 + 1) * 128],
                                 pjt[:, 256:512], start=False, stop=True)
                stats = small.tile([128, 6], F32, tag="stats")
                mv = small.tile([128, 2], F32, tag="mv")
                nc.vector.bn_stats(out=stats[:], in_=pty[:])
                nc.vector.bn_aggr(out=mv[:], in_=stats[:])
                rstd = small.tile([128, 1], F32, tag="rstd")
                negmu = small.tile([128, 1], F32, tag="negmu")
                nc.scalar.activation(out=rstd[:], in_=mv[:, 1:2], func=ACTF.Sqrt,
                                     bias=eps[:], scale=1.0)
                nc.vector.reciprocal(rstd[:], rstd[:])
                nc.vector.tensor_scalar_mul(negmu[:], mv[:, 0:1], -1.0)
                ysl = slice(t2 * 256, (t2 + 1) * 256)
                nc.vector.tensor_scalar(Yall[:, ysl], pty[:], negmu[:], rstd[:],
                                        op0=ALU.add, op1=ALU.mult)
                if apply_gb:
                    nc.vector.tensor_mul(Yall[:, ysl], Yall[:, ysl], gt[:])
                    nc.vector.tensor_add(Yall[:, ysl], Yall[:, ysl], bt[:])

            # ---- S10: one strided store ----
            yv = y_d[:].rearrange("(t r) d -> r t d", r=128)
            nc.scalar.dma_start(out=yv, in_=Yall[:])

    _split_multi_waits(nc)
    return nc


def _get_nc(apply_gb):
    key = bool(apply_gb)
    if key not in _NC_CACHE:
        _NC_CACHE[key] = _build_nc(key)
    return _NC_CACHE[key]


def _make_inputs(x, W_low, W_mid, W_high, proj_w, ln_g, ln_b):
    kh2, cs2, ICS = _host_matrices()

    W_eff = W_high[0].copy()
    W_eff[:32, :32] += W_mid[0]
    W_eff[:16, :16] += W_low[0]
    weffs = []
    for P in range(2):
        wr = W_eff[:, :, P * 128:(P + 1) * 128].transpose(1, 0, 2).reshape(64, 8192)
        weffs.append(np.ascontiguousarray(
            wr.reshape(64, 2, 4096).transpose(1, 0, 2).reshape(128, 4096)
        ))

    pjt = np.zeros((128, 512), ml_dtypes.bfloat16)
    pjt[:, :256] = proj_w.T[:128]
    pjt[:, 256:] = proj_w.T[128:]

    gb = np.stack([ln_g, ln_b]).astype(np.float32)
    consts = {"kh": kh2, "cs": cs2, "ics": ICS,
              "weffa": weffs[0], "weffb": weffs[1], "pjt": pjt, "gb": gb}

    in_maps = []
    for b in range(B):
        m = dict(consts)
        for P, name in ((0, "xra"), (1, "xrb")):
            xp = x[b].reshape(64, 64, 256)[:, :, P * 128:(P + 1) * 128]
            m[name] = np.ascontiguousarray(
                xp.reshape(64, 2, 32, 128).transpose(1, 0, 2, 3).reshape(128, 4096)
            )
        in_maps.append(m)
    return in_maps


def kernel(x, W_low, W_mid, W_high, proj_w, ln_g, ln_b):
    x = np.ascontiguousarray(np.asarray(x, dtype=np.float32))
    W_low = np.asarray(W_low, dtype=np.float32)
    W_mid = np.asarray(W_mid, dtype=np.float32)
    W_high = np.asarray(W_high, dtype=np.float32)
    proj_w = np.asarray(proj_w, dtype=np.float32)
    ln_g = np.asarray(ln_g, dtype=np.float32)
    ln_b = np.asarray(ln_b, dtype=np.float32)

    apply_gb = not (np.all(ln_g == 1.0) and np.all(ln_b == 0.0))
    in_maps = _make_inputs(x, W_low, W_mid, W_high, proj_w, ln_g, ln_b)
    nc = _get_nc(apply_gb)
    res = run_bass_kernel_spmd(nc, in_maps, core_ids=list(range(B)))

    out = np.empty((B, N, C), np.float32)
    for b in range(B):
        yc = res.results[b]["y"]
        out[b] = yc.reshape(64, 64, 256).transpose(1, 0, 2).reshape(4096, 256)
    return out


# revision 11
# speedup vs baseline: 1.3264x; 1.0231x over previous
"""Trainium2 Bass kernel for nn_BandSplitDCTFilter.

Math: the reference's mirror-FFT DCT / band filter / inverse collapses to
    out_c = C1 (Z_c) C2^T - S1 (Z_c) S2^T,   Z_c = (A x_c A^T) .* W_eff_c
with A[k,j] = 2cos(pi k (2j+1)/128); C2/S2 carry the irfft half-spectrum
weights u_l and the 1/(4HW) scale; W_eff = pad(W_low)+pad(W_mid)+W_high
merges the three bands (they share the inverse basis under zero-padding).
Then y = x_out @ proj_w^T and LayerNorm.

Sharding: pure data-parallel, one sample per core (B=8 = 8 cores), small
weights replicated.

Per-core pipeline (v9). 64-row tensors are "packed": free dim split in
half across partition ranges [0:64) and [64:128) so every engine op and
PSUM tile runs 128 partitions wide. Layout pivots ride DRAM (strided
store with >=512B runs + contiguous reload) to keep the DMA instruction
count tiny — this container pays ~0.6-0.8us of sequencer time per DMA
instruction, so many small DMAs serialize.

  S2  F-h : T1[k,(w,c)]   = AT.T @ x[h,(w,c)]      (2 packed halves)
  P1  T1 -> DRAM (k,w,c order) -> T2[w,(k,c)]       (2+2 DMA instrs)
  S4  F-w : Z[l,(k,c)]    = AT.T @ T2, * W_eff fused in drain
  S5  I-l : U2s[(cs,n),(k,c)] = [C2T|S2T].T @ Z     (cos/sin stacked)
  P2  U2s -> DRAM ((cs,k),(n,c) order) -> Ustk      (2+1 DMA instrs, bf16)
  S7  I-k : per (n,chalf): X01[cj, m] = Ustk-chunk.T @ [C1T;-S1T]
  S8  proj: per 128-row tile: Y = X0c.T pjt0 + X1c.T pjt1
  S9  LN  : bn_stats/aggr + fused (y-mu)*rstd -> Yall -> 1 DMA out
Host does layout-only prep (shard/pack) and row unpermute.
"""

import numpy as np
import ml_dtypes

import bass_rust
import concourse.bass as bass
import concourse.mybir as mybir
from concourse.tile import TileContext, ScopedClock
from concourse.bass_utils import run_bass_kernel_spmd

# ---------------------------------------------------------------------------
# Workarounds: this container's walrus rejects >1 sync wait per instruction.
# ---------------------------------------------------------------------------

_wait_ctr = 0


def _split_multi_waits(nc, max_waits=1):
    global _wait_ctr
    for f in nc.m.functions:
        for bb in f.blocks:
            out = []
            dirty = False
            for ins in bb.instructions:
                si = ins.sync_info
                if si is not None and len(si.on_wait) > max_waits:
                    waits = list(si.on_wait)
                    for w in waits[:-max_waits]:
                        _wait_ctr += 1
                        nop = bass_rust.InstNoOp(name=f"I-waitsplit-{_wait_ctr}")
                        nop.engine = ins.engine
                        nop.sync_info = mybir.SyncInfo(on_wait=[w], on_update=[])
                        out.append(nop)
                    ins.sync_info = mybir.SyncInfo(
                        on_wait=waits[-max_waits:], on_update=list(si.on_update)
                    )
                    dirty = True
                out.append(ins)
            if dirty:
                bb.instructions = out


def _patched_drain_and_barrier(self, tick_clock, wait_clock):
    nc = self.nc
    probe = nc.sync.nop(nofuse=True)
    wait_clock.add_sem_waits(probe.ins, ScopedClock({None: tick_clock.global_clock}))
    si = probe.ins.sync_info
    waits = list(si.on_wait) if si is not None else []
    probe.ins.sync_info = mybir.SyncInfo(on_wait=waits[:1], on_update=[])
    name2sem = {s.name: s for s in self.sems.allocated().values()}
    for w in waits[1:]:
        nc.sync.nop(nofuse=True)._wait_ge(name2sem[w.ant_name], w.wait_value)
    nc.sync.drain()
    nc.all_engine_barrier()
    popped = nc._tile_sem_poison_stack.pop()
    assert popped is self._sem_poison
    nc.clear_and_free_semaphores(list(self.sems.allocated().values()))
    nc.all_engine_barrier()


TileContext._drain_and_barrier = _patched_drain_and_barrier

# ---------------------------------------------------------------------------

B, H, W, C = 8, 64, 64, 256
N = H * W
F32 = mybir.dt.float32
F32R = mybir.dt.float32r
BF16 = mybir.dt.bfloat16
ALU = mybir.AluOpType
ACTF = mybir.ActivationFunctionType


def _host_matrices():
    k = np.arange(64)
    j = np.arange(64)
    ang = np.pi * k[:, None] * (2 * j[None, :] + 1) / 128.0
    A = 2.0 * np.cos(ang)
    u = np.where(k == 0, 1.0, 2.0)
    C1T = np.cos(ang)
    S1T = np.sin(ang)
    C2T = u[:, None] * np.cos(ang) / 16384.0
    S2T = u[:, None] * np.sin(ang) / 16384.0

    AT = A.T.astype(np.float32)                                   # [h, k]
    kh2 = np.concatenate([AT, AT], axis=0).astype(np.float32)     # [128, 64]
    cs2_half = np.concatenate([C2T, S2T], axis=1)                 # [l, 128]
    cs2 = np.concatenate([cs2_half, cs2_half], axis=0).astype(np.float32)
    ICS = np.concatenate([C1T, -S1T], axis=0).astype(ml_dtypes.bfloat16)
    return kh2, cs2, np.ascontiguousarray(ICS)


_NC_CACHE = {}


def _build_nc(apply_gb):
    nc = bass.Bass(trn_type="TRN2")

    xa_d = nc.dram_tensor("xra", [128, 4096], F32R, kind="ExternalInput")
    xb_d = nc.dram_tensor("xrb", [128, 4096], F32R, kind="ExternalInput")
    kh_d = nc.dram_tensor("kh", [128, 64], F32R, kind="ExternalInput")
    cs_d = nc.dram_tensor("cs", [128, 128], F32R, kind="ExternalInput")
    ics_d = nc.dram_tensor("ics", [128, 64], BF16, kind="ExternalInput")
    wa_d = nc.dram_tensor("weffa", [128, 4096], F32, kind="ExternalInput")
    wb_d = nc.dram_tensor("weffb", [128, 4096], F32, kind="ExternalInput")
    pjt_d = nc.dram_tensor("pjt", [128, 512], BF16, kind="ExternalInput")
    gb_d = nc.dram_tensor("gb", [2, 256], F32, kind="ExternalInput")
    y_d = nc.dram_tensor("y", [4096, 256], F32, kind="ExternalOutput")

    with TileContext(nc) as tc:
        with (
            tc.tile_pool(name="consts", bufs=1) as consts,
            tc.tile_pool(name="wfA", bufs=1) as wfA,
            tc.tile_pool(name="wfB", bufs=1) as wfB,
            tc.tile_pool(name="sA1", bufs=1) as sA1,
            tc.tile_pool(name="sA2", bufs=1) as sA2,
            tc.tile_pool(name="sA3", bufs=1) as sA3,
            tc.tile_pool(name="sB1", bufs=1) as sB1,
            tc.tile_pool(name="sB2", bufs=1) as sB2,
            tc.tile_pool(name="sB3", bufs=1) as sB3,
            tc.tile_pool(name="zA", bufs=1) as zA,
            tc.tile_pool(name="zB", bufs=1) as zB,
            tc.tile_pool(name="dramp", bufs=1, space="DRAM") as dramp,
            tc.tile_pool(name="ps", bufs=8, space="PSUM") as ps,
            tc.tile_pool(name="small", bufs=8) as small,
        ):
            # ---- constants ----
            kh2 = consts.tile([128, 64], F32R, tag="kh2")
            cs2 = consts.tile([128, 128], F32R, tag="cs2")
            ics = consts.tile([128, 64], BF16, tag="ics")
            pjt = consts.tile([128, 512], BF16, tag="pjt")
            nc.sync.dma_start(out=kh2[:], in_=kh_d[:])
            nc.sync.dma_start(out=cs2[:], in_=cs_d[:])
            nc.gpsimd.dma_start(out=ics[:], in_=ics_d[:])
            nc.gpsimd.dma_start(out=pjt[:], in_=pjt_d[:])
            eps = consts.tile([128, 1], F32, tag="eps")
            nc.vector.memset(eps[:], 1e-5)
            weffA = wfA.tile([128, 4096], F32, tag="wfA")
            weffB = wfB.tile([128, 4096], F32, tag="wfB")
            nc.gpsimd.dma_start(out=weffA[:], in_=wa_d[:])
            nc.gpsimd.dma_start(out=weffB[:], in_=wb_d[:])
            if apply_gb:
                gt = consts.tile([128, 256], F32, tag="gt")
                bt = consts.tile([128, 256], F32, tag="bt")
                gb_ap = gb_d.ap()
                g_b = bass.AP(tensor=gb_ap.tensor, offset=0, ap=[[0, 128], [1, 256]])
                b_b = bass.AP(tensor=gb_ap.tensor, offset=256, ap=[[0, 128], [1, 256]])
                nc.sync.dma_start(out=gt[:], in_=g_b)
                nc.sync.dma_start(out=bt[:], in_=b_b)

            # Two channel-half pipes, emission staggered so one pipe's
            # DRAM pivots overlap the other pipe's compute.
            cfg = {
                0: dict(x_d=xa_d, io=nc.sync, s1=sA1, s2=sA2, s3=sA3, zp=zA),
                1: dict(x_d=xb_d, io=nc.scalar, s1=sB1, s2=sB2, s3=sB3, zp=zB),
            }
            st = {0: {}, 1: {}}

            def s1_load(P):
                c = cfg[P]
                X = c["s1"].tile([128, 4096], F32R, tag=f"s{P}1")
                c["io"].dma_start(out=X[:, 0:2048], in_=c["x_d"][:, 0:2048])
                c["io"].dma_start(out=X[:, 2048:4096], in_=c["x_d"][:, 2048:4096])
                st[P]["X"] = X

            def s2_fh(P):
                c = cfg[P]
                X = st[P]["X"]
                T1p = c["s2"].tile([128, 4096], F32R, tag=f"s{P}2")
                for j in range(16):
                    off = 64 * (j // 8)
                    sl = slice((j % 8) * 512, (j % 8 + 1) * 512)
                    pt = ps.tile([64, 512], F32, tag="ps")
                    nc.tensor.matmul(pt[:], kh2[off:off + 64, :],
                                     X[off:off + 64, sl], start=True, stop=True)
                    eng = nc.vector.tensor_copy if j % 2 == 0 else nc.scalar.copy
                    eng(T1p[off:off + 64, sl], pt[:])
                st[P]["T1p"] = T1p

            def p1_pivot(P):
                c = cfg[P]
                T1p = st[P]["T1p"]
                D1 = dramp.tile([64, 8192], F32R, tag=f"d1{P}")
                D1v = D1[:].rearrange("w (k c) -> k w c", c=128)
                c["io"].dma_start(out=D1v[:, 0:32, :], in_=T1p[0:64, :])
                c["io"].dma_start(out=D1v[:, 32:64, :], in_=T1p[64:128, :])
                T2p = c["s3"].tile([128, 4096], F32R, tag=f"s{P}3")
                c["io"].dma_start(out=T2p[0:64, :], in_=D1[:, 0:4096])
                c["io"].dma_start(out=T2p[64:128, :], in_=D1[:, 4096:8192])
                st[P]["T2p"] = T2p

            def s4_s5(P):
                c = cfg[P]
                T2p = st[P]["T2p"]
                weff = weffA if P == 0 else weffB
                Zp = c["zp"].tile([128, 4096], F32R, tag=f"z{P}")
                for j in range(16):
                    off = 64 * (j // 8)
                    sl = slice((j % 8) * 512, (j % 8 + 1) * 512)
                    pt = ps.tile([64, 512], F32, tag="ps")
                    nc.tensor.matmul(pt[:], kh2[off:off + 64, :],
                                     T2p[off:off + 64, sl], start=True, stop=True)
                    nc.vector.tensor_mul(Zp[off:off + 64, sl], pt[:],
                                         weff[off:off + 64, sl])
                U2s = c["s3"].tile([128, 8192], BF16, tag=f"s{P}3")
                for j in range(16):
                    off = 64 * (j // 8)
                    sl = slice((j % 8) * 512, (j % 8 + 1) * 512)
                    pt = ps.tile([128, 512], F32, tag="ps")
                    nc.tensor.matmul(pt[:], cs2[off:off + 64, :],
                                     Zp[off:off + 64, sl], start=True, stop=True)
                    dsl = slice(j * 512, (j + 1) * 512)
                    eng = nc.vector.tensor_copy if j % 2 == 0 else nc.scalar.copy
                    eng(U2s[:, dsl], pt[:])
                st[P]["U2s"] = U2s

            def p2_pivot(P):
                c = cfg[P]
                U2s = st[P]["U2s"]
                D2 = dramp.tile([128, 8192], BF16, tag=f"d2{P}")
                for cshalf in range(2):
                    dst = D2[cshalf * 64:(cshalf + 1) * 64, :].rearrange(
                        "k (n c) -> n k c", c=128
                    )
                    c["io"].dma_start(out=dst,
                                      in_=U2s[cshalf * 64:(cshalf + 1) * 64, :])
                Ustk = c["s1"].tile([128, 8192], BF16, tag=f"s{P}1")
                c["io"].dma_start(out=Ustk[:], in_=D2[:])
                st[P]["Ustk"] = Ustk

            def s7_ik(P):
                c = cfg[P]
                Ustk = st[P]["Ustk"]
                X01 = c["s2"].tile([128, 4096], BF16, tag=f"s{P}2")
                for g in range(8):
                    pt = ps.tile([128, 512], F32, tag="ps")
                    for nn in range(8):
                        t = 8 * g + nn
                        nc.tensor.matmul(
                            pt[:, nn * 64:(nn + 1) * 64],
                            Ustk[:, t * 128:(t + 1) * 128],
                            ics[:], start=True, stop=True,
                        )
                    eng = nc.vector.tensor_copy if g % 2 == 0 else nc.scalar.copy
                    eng(X01[:, g * 512:(g + 1) * 512], pt[:])
                st[P]["X01"] = X01

            s1_load(0)
            s1_load(1)
            s2_fh(0)
            p1_pivot(0)
            s2_fh(1)
            s4_s5(0)
            p1_pivot(1)
            p2_pivot(0)
            s4_s5(1)
            s7_ik(0)
            p2_pivot(1)
            s7_ik(1)
            X01A, X01B = st[0]["X01"], st[1]["X01"]

            # ---- S8 + S9: proj, LayerNorm -> Yall ----
            Yall = zA.tile([128, 8192], F32, tag="z0")
            for t2 in range(32):
                pty = ps.tile([128, 256], F32, tag="ps")
                nc.tensor.matmul(pty[:], X01A[:, t2 * 128:(t2 + 1) * 128],
                                 pjt[:, 0:256], start=True, stop=False)
                nc.tensor.matmul(pty[:], X01B[:, t2 * 128:(t2 + 1) * 128],
                                 pjt[:, 256:512], start=False, stop=True)
                stats = small.tile([128, 6], F32, tag="stats")
                mv = small.tile([128, 2], F32, tag="mv")
                nc.vector.bn_stats(out=stats[:], in_=pty[:])
                nc.vector.bn_aggr(out=mv[:], in_=stats[:])
                rstd = small.tile([128, 1], F32, tag="rstd")
                negmurstd = small.tile([128, 1], F32, tag="negmurstd")
                nc.scalar.activation(out=rstd[:], in_=mv[:, 1:2], func=ACTF.Sqrt,
                                     bias=eps[:], scale=1.0)
                nc.vector.reciprocal(rstd[:], rstd[:])
                nc.gpsimd.tensor_scalar(negmurstd[:], mv[:, 0:1], rstd[:], -1.0,
                                        op0=ALU.mult, op1=ALU.mult)
                ysl = slice(t2 * 256, (t2 + 1) * 256)
                nc.scalar.activation(out=Yall[:, ysl], in_=pty[:], func=ACTF.Identity,
                                     bias=negmurstd[:], scale=rstd[:])
                if apply_gb:
                    nc.vector.tensor_mul(Yall[:, ysl], Yall[:, ysl], gt[:])
                    nc.vector.tensor_add(Yall[:, ysl], Yall[:, ysl], bt[:])

            # ---- S10: one strided store ----
            yv = y_d[:].rearrange("(t r) d -> r t d", r=128)
            nc.scalar.dma_start(out=yv, in_=Yall[:])

    _split_multi_waits(nc)
    return nc


def _get_nc(apply_gb):
    key = bool(apply_gb)
    if key not in _NC_CACHE:
        _NC_CACHE[key] = _build_nc(key)
    return _NC_CACHE[key]


def _make_inputs(x, W_low, W_mid, W_high, proj_w, ln_g, ln_b):
    kh2, cs2, ICS = _host_matrices()

    W_eff = W_high[0].copy()
    W_eff[:32, :32] += W_mid[0]
    W_eff[:16, :16] += W_low[0]
    weffs = []
    for P in range(2):
        wr = W_eff[:, :, P * 128:(P + 1) * 128].transpose(1, 0, 2).reshape(64, 8192)
        weffs.append(np.ascontiguousarray(
            wr.reshape(64, 2, 4096).transpose(1, 0, 2).reshape(128, 4096)
        ))

    pjt = np.zeros((128, 512), ml_dtypes.bfloat16)
    pjt[:, :256] = proj_w.T[:128]
    pjt[:, 256:] = proj_w.T[128:]

    gb = np.stack([ln_g, ln_b]).astype(np.float32)
    consts = {"kh": kh2, "cs": cs2, "ics": ICS,
              "weffa": weffs[0], "weffb": weffs[1], "pjt": pjt, "gb": gb}

    in_maps = []
    for b in range(B):
        m = dict(consts)
        for P, name in ((0, "xra"), (1, "xrb")):
            xp = x[b].reshape(64, 64, 256)[:, :, P * 128:(P + 1) * 128]
            m[name] = np.ascontiguousarray(
                xp.reshape(64, 2, 32, 128).transpose(1, 0, 2, 3).reshape(128, 4096)
            )
        in_maps.append(m)
    return in_maps


def kernel(x, W_low, W_mid, W_high, proj_w, ln_g, ln_b):
    x = np.ascontiguousarray(np.asarray(x, dtype=np.float32))
    W_low = np.asarray(W_low, dtype=np.float32)
    W_mid = np.asarray(W_mid, dtype=np.float32)
    W_high = np.asarray(W_high, dtype=np.float32)
    proj_w = np.asarray(proj_w, dtype=np.float32)
    ln_g = np.asarray(ln_g, dtype=np.float32)
    ln_b = np.asarray(ln_b, dtype=np.float32)

    apply_gb = not (np.all(ln_g == 1.0) and np.all(ln_b == 0.0))
    in_maps = _make_inputs(x, W_low, W_mid, W_high, proj_w, ln_g, ln_b)
    nc = _get_nc(apply_gb)
    res = run_bass_kernel_spmd(nc, in_maps, core_ids=list(range(B)))

    out = np.empty((B, N, C), np.float32)
    for b in range(B):
        yc = res.results[b]["y"]
        out[b] = yc.reshape(64, 64, 256).transpose(1, 0, 2).reshape(4096, 256)
    return out
